# revision 11
# baseline (speedup 1.0000x reference)
"""CBAM channel attention kernel for Trainium2 (8 NeuronCores, batch-parallel).

x: [32, 768, 56, 56] f32 on host, cast to bf16 for the device pass so HBM
traffic is 2 bytes/elem each way (38.7 MB/core, ~90 us at the ~435 GB/s
per-core DMA fabric) instead of 77.4 MB at f32.  The 2e-2 rel-err gate has
~10x margin over bf16 rounding (~2.5e-3 Frobenius).  Each core handles 4
samples; channel-chunk pairs [128, 2, 3136] stay resident in SBUF between
pooling and scaling, so HBM traffic is exactly 1 read + 1 write of x.

Pooling: max on DVE as a depth-2 tensor_tensor max tree over the pair's
free-dim halves (2x-pumped at bf16) feeding one accumulate-variant
tensor_scalar (CACHE_REDUCE, the only HW-safe reduce+accum op: plain
tensor_reduce costs 6.7us/pair, tensor_tensor_reduce hangs the device on
bf16) per chunk on the 784-wide remnant -- ~4.8us/pair vs 6.7us plain.
Sum on ScalarE (activation Copy + accum_out; the main output streams to a
zero-stride sink AP so pooling never writes the tile).  MLP runs in
transposed form on TensorE with host-pretransposed f32 weights:
hT = w1T.T @ pooledT, exact gelu via Erf, mlpT per chunk, sigmoid from
PSUM.  Gate applied OUT-OF-PLACE into a separate write-tile pool (DVE
tensor_scalar, 4x-pumped at bf16, ~1us/chunk) so the read stream only
waits on the multiply, not on the write DMA drain; otiles run 6 bufs
(2 samples) to force the scheduler into per-sample pool->mult->write
cadence instead of front-loading all pooling.  DMA out: SWDGE for samples
0-2, per-chunk on the Sync+ACT HWDGE rings for the last sample; reads
alternate Sync/ACT rings.
"""

import ml_dtypes
import numpy as np

import concourse.bacc as bacc
import concourse.bass as bass
import concourse.mybir as mybir
import concourse.tile as tile
from concourse.bass_utils import run_bass_kernel_spmd

B = 32
C = 768
HW = 56 * 56    # 3136
HWH = HW // 2   # 1568
HWQ = HW // 4   # 784
HID = 48        # C // 16
NCORES = 8
B_LOC = B // NCORES  # 4
KC = C // 128        # 6 channel chunks
F32 = mybir.dt.float32
BF16 = mybir.dt.bfloat16
AF = mybir.ActivationFunctionType
ALU = mybir.AluOpType

_cache = {}


def _build_nc():
    nc = bacc.Bacc("TRN2", target_bir_lowering=False, debug=False)
    x_d = nc.declare_dram_parameter("x", [B_LOC * C, HW], BF16, isOutput=False)
    # host-pretransposed weights: w1t[p, k, h] = w1[h, k*128+p],
    # w2t[h, k, p] = 0.5 * w2[k*128+p, h]  (0.5 folds the gelu half)
    w1_d = nc.declare_dram_parameter("w1t", [128, KC * HID], F32, isOutput=False)
    w2_d = nc.declare_dram_parameter("w2t", [HID, KC * 128], F32, isOutput=False)
    out_d = nc.declare_dram_parameter("out", [B_LOC * C, HW], BF16, isOutput=True)

    with tile.TileContext(nc) as tc:
        with (
            tc.tile_pool(name="consts", bufs=1) as consts,
            tc.tile_pool(name="otiles", bufs=10) as opool,
            tc.tile_pool(name="wtiles", bufs=4) as wpool,
            tc.tile_pool(name="ttree", bufs=2) as tpool,
            tc.tile_pool(name="pooled", bufs=3) as pooled_pool,
            tc.tile_pool(name="small", bufs=3) as small_pool,
            tc.tile_pool(name="psum", bufs=2, space="PSUM") as psum_pool,
        ):
            # const loads ride the ACT ring so the first pair read leads
            # the Sync queue
            w1T = consts.tile([128, KC, HID], F32)
            nc.scalar.dma_start(
                out=w1T, in_=w1_d.rearrange("p (k h) -> p k h", k=KC)
            )
            w2T = consts.tile([HID, KC, 128], F32)
            nc.scalar.dma_start(
                out=w2T, in_=w2_d.rearrange("h (k p) -> h k p", k=KC)
            )

            sink = consts.tile([128, 1], BF16)
            # write-only scratch for the CACHE_REDUCE output streams (WAW
            # chains are free: all writers sit in-order on the DVE queue)
            garbage = consts.tile([128, HWQ], BF16)

            for b in range(B_LOC):
                ots = []
                pooled = pooled_pool.tile([128, KC, 2], F32)
                for j in range(KC // 2):
                    # chunk pairs ride one 1.6MB DMA; reads alternate the two
                    # HWDGE rings so neither queue caps the stream
                    ot = opool.tile([128, 2, HW], BF16, tag="o")
                    row = (b * KC + 2 * j) * 128
                    reng = nc.sync if j != 1 else nc.scalar
                    reng.dma_start(
                        out=ot,
                        in_=x_d[row : row + 256, :].rearrange(
                            "(k p) f -> p k f", p=128
                        ),
                    )
                    # max: depth-2 TT-max tree over free-dim halves (both
                    # chunks per op, 2x at bf16), then one CACHE_REDUCE per
                    # chunk on the 784-wide remnant
                    t1 = tpool.tile([128, 2, HWH], BF16, tag="t1")
                    nc.vector.tensor_tensor(
                        out=t1, in0=ot[:, :, 0:HWH], in1=ot[:, :, HWH:HW],
                        op=ALU.max,
                    )
                    t2 = tpool.tile([128, 2, HWQ], BF16, tag="t2")
                    nc.vector.tensor_tensor(
                        out=t2, in0=t1[:, :, 0:HWQ], in1=t1[:, :, HWQ:HWH],
                        op=ALU.max,
                    )
                    for i in range(2):
                        k = 2 * j + i
                        nc.vector.tensor_scalar(
                            out=garbage[:, 0:HWQ],
                            in0=t2[:, i, :],
                            scalar1=0.0,
                            scalar2=None,
                            op0=ALU.bypass,
                            op1=ALU.max,
                            accum_out=pooled[:, k, 1:2],
                        )
                        # sum: ACT Copy + accum, main output to a
                        # zero-stride sink (~3.2us/chunk, ~80us total --
                        # just under the ~89us DMA floor)
                        nc.scalar.activation(
                            out=sink[:, 0:1].to_broadcast([128, HW]),
                            in_=ot[:, i, :],
                            func=AF.Copy,
                            accum_out=pooled[:, k, 0:1],
                        )
                    ots.append(ot)

                # hT [48, 2] = sum_k w1T_k.T @ pooledT_k
                hps = psum_pool.tile([HID, 2], F32, tag="hps")
                for k in range(KC):
                    nc.tensor.matmul(
                        hps,
                        w1T[:, k, :],
                        pooled[:, k, :],
                        start=(k == 0),
                        stop=(k == KC - 1),
                    )
                # avg column holds the raw sum; scale to the mean here (cheaper
                # than scaling 6 [128,1] pooled slots or a [128,3136] tile)
                nc.vector.tensor_scalar_mul(hps[:, 0:1], hps[:, 0:1], 1.0 / HW)
                e_sb = small_pool.tile([HID, 2], F32, tag="e")
                nc.scalar.activation(
                    out=e_sb, in_=hps, func=AF.Erf, scale=0.7071067811865476
                )
                # hh' = (e + 1) * u   (u = pre-gelu matmul output); the gate
                # path is linear in hh, so accum_out sums avg+max columns
                # directly into hsum for matmul2
                hh = small_pool.tile([HID, 2], F32, tag="hh")
                hsum = small_pool.tile([HID, 1], F32, tag="hsum")
                nc.vector.scalar_tensor_tensor(
                    out=hh, in0=e_sb, scalar=1.0, in1=hps,
                    op0=ALU.add, op1=ALU.mult, accum_out=hsum,
                )
                mlp = psum_pool.tile([128, KC], F32, tag="mlp")
                for k in range(KC):
                    nc.tensor.matmul(
                        mlp[:, k : k + 1],
                        w2T[:, k, :],
                        hsum,
                        start=True,
                        stop=True,
                    )
                gate = small_pool.tile([128, KC], F32, tag="gate")
                nc.scalar.activation(out=gate, in_=mlp, func=AF.Sigmoid)

                for j in range(KC // 2):
                    ot = ots[j]
                    row = (b * KC + 2 * j) * 128
                    # gate-multiplies OUT-OF-PLACE into write tiles (DVE,
                    # 4x-pumped at bf16) so ot frees for the next sample's
                    # read as soon as the multiply has consumed it; the last
                    # sample hands chunk 4 to the now-idle ACT
                    wt = wpool.tile([128, 2, HW], BF16, tag="w")
                    for i in range(2):
                        k = 2 * j + i
                        if b == B_LOC - 1 and k == 4:
                            nc.scalar.activation(
                                out=wt[:, i, :], in_=ot[:, i, :], func=AF.Copy,
                                scale=gate[:, k : k + 1],
                            )
                        else:
                            nc.vector.tensor_scalar_mul(
                                wt[:, i, :], ot[:, i, :], gate[:, k : k + 1]
                            )
                    # writes ride SWDGE (GpSimd) so they never head-of-line
                    # block the read FIFO on the HWDGE rings; the last
                    # sample's writes go on the (now idle) HWDGE rings
                    # instead, letting the POOL dge_drain start early
                    if b == B_LOC - 1:
                        for i in range(2):
                            eng = nc.sync if i == 0 else nc.scalar
                            eng.dma_start(
                                out=out_d[row + 128 * i : row + 128 * (i + 1), :],
                                in_=wt[:, i, :],
                            )
                    else:
                        out_ap = out_d[row : row + 256, :].rearrange(
                            "(k p) f -> p k f", p=128
                        )
                        nc.gpsimd.dma_start(out=out_ap, in_=wt)
    nc.finalize()
    return nc


def kernel(x, w1, w2, _trace=False):
    if "nc" not in _cache:
        _cache["nc"] = _build_nc()
    nc = _cache["nc"]

    x = np.asarray(x).reshape(B, C, HW)
    w1t = np.ascontiguousarray(
        np.asarray(w1, np.float32).reshape(HID, KC, 128).transpose(2, 1, 0)
        .reshape(128, KC * HID)
    )
    w2t = np.ascontiguousarray(
        (0.5 * np.asarray(w2, np.float32)).reshape(KC, 128, HID)
        .transpose(2, 0, 1).reshape(HID, KC * 128)
    )
    in_maps = [
        {
            "x": np.ascontiguousarray(
                x[i * B_LOC : (i + 1) * B_LOC].reshape(B_LOC * C, HW)
            ).astype(ml_dtypes.bfloat16),
            "w1t": w1t,
            "w2t": w2t,
        }
        for i in range(NCORES)
    ]
    res = run_bass_kernel_spmd(nc, in_maps, core_ids=list(range(NCORES)),
                               trace=_trace)
    out = np.concatenate(
        [
            r["out"].astype(np.float32).reshape(B_LOC, C, 56, 56)
            for r in res.results
        ],
        axis=0,
    )
    if _trace:
        _cache["last_results"] = res
    return out
